# revision 10
# baseline (speedup 1.0000x reference)
"""Single attention head (B=8, S=2048, D=768, H=12) on 8 TRN2 NeuronCores.

Strategy: data-parallel over batch (1 batch element per core). Host-side
prep is layout only: per-batch permutation placing masked-in keys first
(compacts the key extent from 2048 to ~1152), x transposed to (D, S) so the
projection contraction dim lands on SBUF partitions, weights packed as
[Wk | Wq/sqrt(H) | Wv], mask converted to an additive bias row.

Device pipeline per core:
  1. qkvT (36, S) = W_cat^T @ xT               (TensorE, fp32)
  2. pass A (s-part):  scores = q.k + bias      -> DVE reduce_max -> m_s
     (bias folded in via an extra contraction row: ones x bias)
  3. pass B (t-part):  q.k + bias - m_s         (bias and m folded in as two
     extra contraction rows) -> ACT exp from PSUM -> bf16 pT
  4. out_aug = [v | 1]^T @ pT  (softmax denominator = row 12 for free)
     -> transpose 128-blocks back to s-part, multiply by reciprocal of sums
"""

import math
import os

import numpy as np

B, S, D, H = 8, 2048, 768, 12
N_CORES = 8
F32MAX_NEG = -1.0e30


def _build(nc_mod, T_pad):
    """Build the Bass graph for one core (SPMD across 8)."""
    bass, mybir, tile, bacc = nc_mod
    f32 = mybir.dt.float32
    bf16 = mybir.dt.bfloat16
    AF = mybir.ActivationFunctionType
    X = mybir.AxisListType.X

    NT = T_pad // 128           # t tiles
    HALF = T_pad // 2           # A-pass half extent (multiple of 64)
    NCH = 4                     # s chunks
    SCH = S // NCH              # 512
    STC = SCH // 128            # s tiles per chunk = 4

    nc = bacc.Bacc("TRN2", target_bir_lowering=False, debug=False,
                   num_devices=N_CORES)

    xT_ext = nc.dram_tensor("xT", [D, S], f32, kind="ExternalInput")
    w_ext = nc.dram_tensor("w", [D, 96], f32, kind="ExternalInput")
    bias_ext = nc.dram_tensor("biasrow", [1, T_pad], f32, kind="ExternalInput")
    out_ext = nc.dram_tensor("out", [128, 256], f32, kind="ExternalOutput")

    from concourse.masks import make_identity

    with tile.TileContext(nc) as tc:
        with tc.tile_pool(name="sb", bufs=1) as sb, \
             tc.tile_pool(name="tmp", bufs=2) as tmp, \
             tc.tile_pool(name="pt", bufs=4) as ptp:

            xT = sb.tile([128, 6, S], f32)
            w = sb.tile([128, 6, 96], f32)
            # rows 0-11 kT, 12 bias, 13 = -1.0 ; replica at rows 32-45
            kTb = sb.tile([128, T_pad], f32)
            # rows 0-11 qT, 12 = +1.0, 13 = m ; replica at rows 32-45
            rhsB = sb.tile([128, S], f32)
            vTsb = sb.tile([12, T_pad], f32)
            vaug = sb.tile([128, NT, 16], bf16)
            ident = sb.tile([128, 128], f32)
            maxh = sb.tile([128, 2, 16], f32)   # per-half maxes
            maxc = sb.tile([128, 16], f32)      # combined max per s tile
            negst = sb.tile([1, 2, SCH], f32)   # negm staging rows (x2 chunks)
            vaccs = sb.tile([13, 2, SCH], f32)  # V output staging (x2 chunks)
            outsb = sb.tile([128, 16, 16], f32)

            # ---- fills (full-tile: gpsimd memset needs partition 0 start)
            nc.gpsimd.memset(kTb[:, :], -1.0)   # row 13 stays -1.0
            nc.gpsimd.memset(rhsB[:, :], 1.0)   # row 12 stays +1.0

            # ---- input DMAs ----
            nc.sync.dma_start(w[:], w_ext.ap().rearrange(
                "(ko p) m -> p ko m", p=128))
            nc.sync.dma_start(kTb[12:13, :], bias_ext.ap())
            xTr = xT_ext.ap().rearrange("(ko p) s -> p ko s", p=128)
            for c in range(NCH):
                cs = slice(c * SCH, (c + 1) * SCH)
                nc.sync.dma_start(xT[:, :, cs], xTr[:, :, cs])

            make_identity(nc, ident[:])
            nc.gpsimd.memset(vaug[:, :, 12:16], 0.0)
            nc.gpsimd.memset(vaug[:, :, 12:13], 1.0)

            # ---- projections ----
            ncov = (T_pad + SCH - 1) // SCH   # chunks covering the key range
            with tc.tile_pool(name="projp", bufs=1, space="PSUM") as projp:
                qkv = projp.tile([96, S], f32)
                for c in range(NCH):
                    cs = slice(c * SCH, (c + 1) * SCH)
                    for ko in range(6):
                        nc.tensor.matmul(
                            qkv[0:96, cs], w[:, ko, :], xT[:, ko, cs],
                            start=(ko == 0), stop=(ko == 5))
                    # qT egress for this chunk + packing replica
                    nc.scalar.copy(rhsB[0:12, cs], qkv[32:44, cs])
                    nc.sync.dma_start(rhsB[32:45, cs], rhsB[0:13, cs])
                    if c == ncov - 1:
                        nc.scalar.copy(kTb[0:12, :], qkv[0:12, 0:T_pad])
                        nc.scalar.copy(vTsb[:], qkv[64:76, 0:T_pad])
                        nc.sync.dma_start(kTb[32:46, :], kTb[0:14, :])

            apool_free = max(1024, HALF)
            with tc.tile_pool(name="Ap", bufs=2, space="PSUM") as Ap, \
                 tc.tile_pool(name="smp", bufs=3, space="PSUM") as smp, \
                 tc.tile_pool(name="vap", bufs=1, space="PSUM") as vap:

                # v tiles -> (t-part, h) layout with ones column
                for j in range(NT):
                    trv = smp.tile([128, 512], f32, tag="sm")
                    nc.tensor.transpose(
                        trv[0:128, 0:12], vTsb[:, j * 128:(j + 1) * 128],
                        ident[0:12, 0:12])
                    nc.vector.tensor_copy(vaug[:, j, 0:12], trv[0:128, 0:12])

                def emit_A_mm(c):
                    """A-pass matmuls + row maxes for s chunk c (2-way packed)."""
                    for pr in range(STC // 2):
                        st0 = c * STC + 2 * pr
                        st1 = st0 + 1
                        s0 = slice(st0 * 128, (st0 + 1) * 128)
                        s1 = slice(st1 * 128, (st1 + 1) * 128)
                        for h in range(2):
                            t0 = Ap.tile([128, apool_free], f32, tag="A")
                            t1 = Ap.tile([128, apool_free], f32, tag="A")
                            base = h * HALF
                            for off in range(0, HALF, 512):
                                n = min(512, HALF - off)
                                tsl = slice(base + off, base + off + n)
                                nc.tensor.matmul(
                                    t0[:, off:off + n], rhsB[0:13, s0],
                                    kTb[0:13, tsl], start=True, stop=True,
                                    tile_position=(0, 0))
                                nc.tensor.matmul(
                                    t1[:, off:off + n], rhsB[32:45, s1],
                                    kTb[32:45, tsl], start=True, stop=True,
                                    tile_position=(32, 0))
                            nc.vector.reduce_max(
                                maxh[:, h, st0:st0 + 1], t0[:, 0:HALF], axis=X)
                            nc.vector.reduce_max(
                                maxh[:, h, st1:st1 + 1], t1[:, 0:HALF], axis=X)
                    c4 = slice(c * STC, (c + 1) * STC)
                    nc.vector.tensor_max(
                        maxc[:, c4], maxh[:, 0, c4], maxh[:, 1, c4])

                def emit_negm(c):
                    """Transpose maxes into row 13 of rhsB for chunk c."""
                    mt = smp.tile([128, 512], f32, tag="sm")
                    for k in range(STC):
                        st = c * STC + k
                        nc.tensor.transpose(
                            mt[0:1, k * 128:(k + 1) * 128],
                            maxc[:, st:st + 1], ident[:])
                    cs = slice(c * SCH, (c + 1) * SCH)
                    nc.scalar.copy(negst[:, c % 2, :], mt[0:1, 0:SCH])
                    nc.sync.dma_start(rhsB[13:14, cs], negst[:, c % 2, :])
                    nc.sync.dma_start(rhsB[45:46, cs], negst[:, c % 2, :])

                def emit_B(c):
                    """scores^T -> exp -> V accumulate for s chunk c."""
                    cs = slice(c * SCH, (c + 1) * SCH)
                    vacc = vap.tile([13, SCH], f32)
                    for jp in range((NT + 1) // 2):
                        j0, j1 = 2 * jp, 2 * jp + 1
                        bt = []
                        for j, rb, tp in ((j0, 0, (0, 0)), (j1, 32, (32, 0))):
                            if j >= NT:
                                continue
                            bp = smp.tile([128, 512], f32, tag="sm")
                            tsl = slice(j * 128, (j + 1) * 128)
                            nc.tensor.matmul(
                                bp[:, 0:SCH], kTb[rb:rb + 14, tsl],
                                rhsB[rb:rb + 14, cs], start=True, stop=True,
                                tile_position=tp)
                            bt.append((j, bp))
                        for j, bp in bt:
                            p = ptp.tile([128, SCH], bf16, tag="p")
                            nc.scalar.activation(p[:], bp[:, 0:SCH], AF.Exp)
                            nc.tensor.matmul(
                                vacc[0:13, :], vaug[:, j, 0:13], p[:],
                                start=(j == 0), stop=(j == NT - 1))
                    nc.scalar.copy(vaccs[:, c % 2, :], vacc[0:13, :])

                def emit_out(c):
                    """Transpose V output back to s-part and normalize."""
                    tro = smp.tile([128, 512], f32, tag="sm")
                    for k in range(STC):
                        nc.tensor.transpose(
                            tro[:, 16 * k:16 * k + 13],
                            vaccs[:, c % 2, k * 128:(k + 1) * 128],
                            ident[0:13, 0:13])
                    trr = tro[:].rearrange("p (k x) -> p k x", x=16)
                    rec = tmp.tile([128, 4], f32, tag="rec")
                    recb = tmp.tile([128, 4, 16], f32, tag="recb")
                    nc.vector.reciprocal(rec[:], trr[:, 0:4, 12])
                    nc.vector.tensor_copy(
                        recb[:], rec[:, :, None].to_broadcast([128, 4, 16]))
                    nc.vector.tensor_mul(
                        outsb[:, c * STC:(c + 1) * STC, :],
                        trr[:, 0:4, :], recb[:])

                emit_A_mm(0)
                emit_negm(0)
                for c in range(NCH):
                    if c + 1 < NCH:
                        emit_A_mm(c + 1)
                    emit_B(c)
                    if c + 1 < NCH:
                        emit_negm(c + 1)
                    if c >= 1:
                        emit_out(c - 1)
                emit_out(NCH - 1)

            nc.sync.dma_start(
                out_ext.ap(), outsb[:].rearrange("p a b -> p (a b)"))

    nc.compile()
    return nc


def kernel(x, mask, key_weight, query_weight, value_weight):
    import concourse.bass as bass
    import concourse.mybir as mybir
    import concourse.tile as tile
    from concourse import bacc, bass_utils

    x = np.asarray(x, dtype=np.float32)
    mask = np.asarray(mask)
    wk = np.asarray(key_weight, dtype=np.float32)
    wq = np.asarray(query_weight, dtype=np.float32)
    wv = np.asarray(value_weight, dtype=np.float32)

    scale = 1.0 / math.sqrt(H)
    w_cat = np.zeros((D, 96), dtype=np.float32)
    w_cat[:, 0:12] = wk          # kT -> psum partitions 0-11
    w_cat[:, 32:44] = wq * scale  # qT -> 32-43
    w_cat[:, 64:76] = wv         # vT -> 64-75

    perms, nbs = [], []
    for b in range(B):
        m = mask[b, 0].astype(np.int64)
        perm = np.argsort(1 - m, kind="stable")
        perms.append(perm)
        nbs.append(int(m.sum()))
    T_pad = max(128, int(np.ceil(max(max(nbs), 1) / 128.0)) * 128)
    T_pad = min(T_pad, S)

    in_maps = []
    for b in range(B):
        xTp = np.ascontiguousarray(x[b].T[:, perms[b]])
        biasrow = np.zeros((1, T_pad), dtype=np.float32)
        biasrow[0, nbs[b]:] = F32MAX_NEG
        in_maps.append({"xT": xTp, "w": w_cat, "biasrow": biasrow})

    import time as _time
    _t0 = _time.time()
    print(f"[kernel] building graph, T_pad={T_pad}", flush=True)
    nc = _build((bass, mybir, tile, bacc), T_pad)
    print(f"[kernel] graph+bacc compile done in {_time.time() - _t0:.1f}s",
          flush=True)

    trace = os.environ.get("BASS_KERNEL_TRACE", "0") == "1"
    if trace:
        import sys
        import types
        from trn_agent_boot.trn_boot import _ntff_profile_via_ctypes
        hook = _ntff_profile_via_ctypes("/opt/axon/libaxon_pjrt.so")
        m = types.ModuleType("antenv.axon_hooks")
        m.get_axon_ntff_profile_hook = lambda: hook
        sys.modules["antenv.axon_hooks"] = m
        bass_utils.upload_artifacts = lambda tmpdir: "local://" + tmpdir

    res = bass_utils.run_bass_kernel_spmd(
        nc, in_maps, core_ids=list(range(N_CORES)), trace=trace)
    if trace:
        print(f"HW exec time: {res.exec_time_ns} ns", flush=True)

    out = np.empty((B, S, H), dtype=np.float32)
    for b in range(B):
        o = res.results[b]["out"].reshape(128, 16, 16)[:, :, :H]
        out[b, perms[b], :] = o.transpose(1, 0, 2).reshape(S, H)
    return out


# revision 14
# speedup vs baseline: 1.9474x; 1.9474x over previous
"""Single attention head (B=8, S=2048, D=768, H=12) on 8 TRN2 NeuronCores.

Data-parallel over batch (1 element/core). Host prep is layout only:
  - per-batch permutation placing masked-in keys first (key extent compacts
    from 2048 to T_pad ~ 1152),
  - x transposed to (D, S) and split into fp16 hi/lo limbs (x scaled by 16
    so limb residuals stay in fp16 normal range),
  - weights packed [Wk | Wq/sqrt(H) | Wv] at 32-aligned columns, scaled by
    64 and split into fp16 limbs (products carry 2^10; descaled on egress),
  - mask converted to an additive fp16 bias row (0 / -60000).

Device pipeline per core (all matmuls fp16-rate; fp32 matmuls on TRN2 are
~4x slower because the compiler splits them into hi/lo passes):
  1. qkvT (96p, S) = 3 limb passes of W^T @ xT accumulated in PSUM.
  2. pass A (s-part): scores_hi = qh.kh + bias via an extra contraction row
     -> DVE reduce_max -> row max m (only needs +-85 accuracy).
  3. pass B (t-part): qh.kh + ql.kh + qh.kl + bias - m, all five terms as
     38 stacked contraction rows in ONE matmul -> ACT exp from PSUM -> fp16
     pT tiles.
  4. out_aug = [v | 1]^T @ pT (softmax denominator free in row 12)
     -> transpose 128-blocks back to s-part, multiply by reciprocal of sums.
"""

import math
import os

import numpy as np

B, S, D, H = 8, 2048, 768, 12
N_CORES = 8
BIAS_NEG = -60000.0
DS = 2.0 ** -10   # descale after limb matmuls (x*16, w*64)


def _build(nc_mod, T_pad):
    bass, mybir, tile, bacc = nc_mod
    f32 = mybir.dt.float32
    f16 = mybir.dt.float16
    AF = mybir.ActivationFunctionType
    OP = mybir.AluOpType
    X = mybir.AxisListType.X

    NT = T_pad // 128           # t tiles
    HALF = T_pad // 2           # A-pass half extent (multiple of 64)
    NCH = 4                     # s chunks
    SCH = S // NCH              # 512
    STC = SCH // 128            # s tiles per chunk = 4

    nc = bacc.Bacc("TRN2", target_bir_lowering=False, debug=False,
                   num_devices=N_CORES)

    xh_ext = nc.dram_tensor("xh", [D, S], f16, kind="ExternalInput")
    xl_ext = nc.dram_tensor("xl", [D, S], f16, kind="ExternalInput")
    w_ext = nc.dram_tensor("w", [D, 192], f16, kind="ExternalInput")
    bias_ext = nc.dram_tensor("biasrow", [1, T_pad], f16, kind="ExternalInput")
    out_ext = nc.dram_tensor("out", [128, 256], f32, kind="ExternalOutput")

    from concourse.masks import make_identity

    with tile.TileContext(nc) as tc:
        with tc.tile_pool(name="sb", bufs=1) as sb, \
             tc.tile_pool(name="tmp", bufs=2) as tmp, \
             tc.tile_pool(name="pt", bufs=4) as ptp:

            xh = sb.tile([128, 6, S], f16)
            xl = sb.tile([128, 6, S], f16)
            w = sb.tile([128, 6, 192], f16)   # [wh(96) | wl(96)] per k-tile
            # kTb rows: 0-11 kh, 12 bias, 13 -1, 14-25 kh dup, 26-37 kl,
            #           64-101 replica of 0-37
            kTb = sb.tile([128, T_pad], f16)
            # rhsB rows: 0-11 qh, 12 +1, 13 m, 14-25 ql, 26-37 qh dup,
            #           64-101 replica of 0-37
            rhsB = sb.tile([128, S], f16)
            qlst = sb.tile([12, S], f16)      # DVE staging for ql
            klst = sb.tile([12, T_pad], f16)  # DVE staging for kl
            vTsb = sb.tile([12, T_pad], f32)
            vaug = sb.tile([128, NT, 16], f16)
            ident = sb.tile([128, 128], f32)
            maxh = sb.tile([128, 2, 16], f32)
            maxc = sb.tile([128, 16], f32)
            negst = sb.tile([1, 2, SCH], f16)
            vaccs = sb.tile([16, 2, SCH], f32)
            outsb = sb.tile([128, 16, 16], f32)

            nc.gpsimd.memset(kTb[:, :], -1.0)   # row 13 stays -1.0
            nc.gpsimd.memset(rhsB[:, :], 1.0)   # rows 12 / 76 stay +1.0
            nc.gpsimd.memset(vaccs[:], 0.0)     # rows 13-15 stay 0

            nc.sync.dma_start(w[:], w_ext.ap().rearrange(
                "(ko p) m -> p ko m", p=128))
            nc.sync.dma_start(kTb[12:13, :], bias_ext.ap())
            xhr = xh_ext.ap().rearrange("(ko p) s -> p ko s", p=128)
            xlr = xl_ext.ap().rearrange("(ko p) s -> p ko s", p=128)
            for c in range(NCH):
                cs = slice(c * SCH, (c + 1) * SCH)
                nc.sync.dma_start(xh[:, :, cs], xhr[:, :, cs])
                nc.sync.dma_start(xl[:, :, cs], xlr[:, :, cs])

            make_identity(nc, ident[:])
            nc.gpsimd.memset(vaug[:, :, 12:16], 0.0)
            nc.gpsimd.memset(vaug[:, :, 12:13], 1.0)

            ncov = (T_pad + SCH - 1) // SCH
            with tc.tile_pool(name="projp", bufs=1, space="PSUM") as projp:
                qkv = projp.tile([96, S], f32)
                for c in range(NCH):
                    cs = slice(c * SCH, (c + 1) * SCH)
                    for ps in range(3):   # wh*xh, wl*xh, wh*xl
                        wsl = slice(96, 192) if ps == 1 else slice(0, 96)
                        xin = xl if ps == 2 else xh
                        for ko in range(6):
                            nc.tensor.matmul(
                                qkv[0:96, cs], w[:, ko, wsl], xin[:, ko, cs],
                                start=(ps == 0 and ko == 0),
                                stop=(ps == 2 and ko == 5))
                    # qh / ql egress for this chunk (+1024 descale)
                    nc.scalar.mul(rhsB[0:12, cs], qkv[32:44, cs], DS)
                    nc.vector.scalar_tensor_tensor(
                        qlst[:, cs], qkv[32:44, cs], DS, rhsB[0:12, cs],
                        op0=OP.mult, op1=OP.subtract)
                    nc.sync.dma_start(rhsB[14:26, cs], qlst[:, cs])
                    nc.sync.dma_start(rhsB[26:38, cs], rhsB[0:12, cs])
                    # base-64 replica rows (A lhsT + B rhs packing)
                    nc.sync.dma_start(rhsB[64:76, cs], rhsB[0:12, cs])
                    nc.sync.dma_start(rhsB[78:90, cs], qlst[:, cs])
                    nc.sync.dma_start(rhsB[90:102, cs], rhsB[0:12, cs])
                    if c == ncov - 1:
                        nc.scalar.mul(kTb[0:12, :], qkv[0:12, 0:T_pad], DS)
                        nc.vector.scalar_tensor_tensor(
                            klst[:], qkv[0:12, 0:T_pad], DS, kTb[0:12, :],
                            op0=OP.mult, op1=OP.subtract)
                        nc.scalar.mul(vTsb[:], qkv[64:76, 0:T_pad], DS)
                        nc.sync.dma_start(kTb[14:26, :], kTb[0:12, :])
                        nc.sync.dma_start(kTb[26:38, :], klst[:])
                        nc.sync.dma_start(kTb[64:102, :], kTb[0:38, :])

            with tc.tile_pool(name="Ap", bufs=2, space="PSUM") as Ap, \
                 tc.tile_pool(name="smp", bufs=3, space="PSUM") as smp, \
                 tc.tile_pool(name="vap", bufs=1, space="PSUM") as vap:

                for j in range(NT):
                    trv = smp.tile([128, 512], f32, tag="sm")
                    nc.tensor.transpose(
                        trv[0:128, 0:12], vTsb[:, j * 128:(j + 1) * 128],
                        ident[0:12, 0:12])
                    nc.vector.tensor_copy(vaug[:, j, 0:12], trv[0:128, 0:12])

                def emit_A_mm(c):
                    for pr in range(STC // 2):
                        st0 = c * STC + 2 * pr
                        st1 = st0 + 1
                        s0 = slice(st0 * 128, (st0 + 1) * 128)
                        s1 = slice(st1 * 128, (st1 + 1) * 128)
                        for h in range(2):
                            t0 = Ap.tile([128, 1024], f32, tag="A")
                            t1 = Ap.tile([128, 1024], f32, tag="A")
                            base = h * HALF
                            for off in range(0, HALF, 512):
                                n = min(512, HALF - off)
                                tsl = slice(base + off, base + off + n)
                                nc.tensor.matmul(
                                    t0[:, off:off + n], rhsB[0:13, s0],
                                    kTb[0:13, tsl], start=True, stop=True,
                                    tile_position=(0, 0))
                                nc.tensor.matmul(
                                    t1[:, off:off + n], rhsB[64:77, s1],
                                    kTb[64:77, tsl], start=True, stop=True,
                                    tile_position=(64, 0))
                            nc.vector.reduce_max(
                                maxh[:, h, st0:st0 + 1], t0[:, 0:HALF], axis=X)
                            nc.vector.reduce_max(
                                maxh[:, h, st1:st1 + 1], t1[:, 0:HALF], axis=X)
                    c4 = slice(c * STC, (c + 1) * STC)
                    nc.vector.tensor_max(
                        maxc[:, c4], maxh[:, 0, c4], maxh[:, 1, c4])

                def emit_negm(c):
                    mt = smp.tile([128, 512], f32, tag="sm")
                    for k in range(STC):
                        st = c * STC + k
                        nc.tensor.transpose(
                            mt[0:1, k * 128:(k + 1) * 128],
                            maxc[:, st:st + 1], ident[:])
                    cs = slice(c * SCH, (c + 1) * SCH)
                    nc.scalar.copy(negst[:, c % 2, :], mt[0:1, 0:SCH])
                    nc.sync.dma_start(rhsB[13:14, cs], negst[:, c % 2, :])
                    nc.sync.dma_start(rhsB[77:78, cs], negst[:, c % 2, :])

                def emit_B(c):
                    cs = slice(c * SCH, (c + 1) * SCH)
                    vacc = vap.tile([13, SCH], f32)
                    for jp in range((NT + 1) // 2):
                        j0, j1 = 2 * jp, 2 * jp + 1
                        bt = []
                        for j, rb, tp in ((j0, 0, (0, 0)), (j1, 64, (64, 0))):
                            if j >= NT:
                                continue
                            bp = smp.tile([128, 512], f32, tag="sm")
                            tsl = slice(j * 128, (j + 1) * 128)
                            nc.tensor.matmul(
                                bp[:, 0:SCH], kTb[rb:rb + 38, tsl],
                                rhsB[rb:rb + 38, cs], start=True, stop=True,
                                tile_position=tp)
                            bt.append((j, bp))
                        for j, bp in bt:
                            p = ptp.tile([128, SCH], f16, tag="p")
                            nc.scalar.activation(p[:], bp[:, 0:SCH], AF.Exp)
                            nc.tensor.matmul(
                                vacc[0:13, :], vaug[:, j, 0:13], p[:],
                                start=(j == 0), stop=(j == NT - 1))
                    nc.scalar.copy(vaccs[0:13, c % 2, :], vacc[0:13, :])

                def emit_out(c):
                    tro = smp.tile([128, 512], f32, tag="sm")
                    for k in range(STC):
                        nc.tensor.transpose(
                            tro[:, 16 * k:16 * k + 16],
                            vaccs[:, c % 2, k * 128:(k + 1) * 128],
                            ident[0:16, 0:16])
                    trr = tro[:].rearrange("p (k x) -> p k x", x=16)
                    rec = tmp.tile([128, 4], f32, tag="rec")
                    recb = tmp.tile([128, 4, 16], f32, tag="recb")
                    nc.vector.reciprocal(rec[:], trr[:, 0:4, 12])
                    nc.vector.tensor_copy(
                        recb[:], rec[:, :, None].to_broadcast([128, 4, 16]))
                    nc.vector.tensor_mul(
                        outsb[:, c * STC:(c + 1) * STC, :],
                        trr[:, 0:4, :], recb[:])

                emit_A_mm(0)
                emit_negm(0)
                for c in range(NCH):
                    if c + 1 < NCH:
                        emit_A_mm(c + 1)
                    emit_B(c)
                    if c + 1 < NCH:
                        emit_negm(c + 1)
                    if c >= 1:
                        emit_out(c - 1)
                emit_out(NCH - 1)

            nc.sync.dma_start(
                out_ext.ap(), outsb[:].rearrange("p a b -> p (a b)"))

    nc.compile()
    return nc


def kernel(x, mask, key_weight, query_weight, value_weight):
    import concourse.bass as bass
    import concourse.mybir as mybir
    import concourse.tile as tile
    from concourse import bacc, bass_utils

    x = np.asarray(x, dtype=np.float32)
    mask = np.asarray(mask)
    wk = np.asarray(key_weight, dtype=np.float32)
    wq = np.asarray(query_weight, dtype=np.float32)
    wv = np.asarray(value_weight, dtype=np.float32)

    # natural-units W, 32-aligned columns, x64 scale for fp16 limb split
    w2 = np.zeros((D, 96), dtype=np.float32)
    w2[:, 0:12] = wk
    w2[:, 32:44] = wq / math.sqrt(H)
    w2[:, 64:76] = wv
    w2 *= 64.0
    wh = w2.astype(np.float16)
    wl = (w2 - wh.astype(np.float32)).astype(np.float16)
    w_cat = np.concatenate([wh, wl], axis=1)  # (768, 192) fp16

    perms, nbs = [], []
    for b in range(B):
        m = mask[b, 0].astype(np.int64)
        perm = np.argsort(1 - m, kind="stable")
        perms.append(perm)
        nbs.append(int(m.sum()))
    T_pad = max(128, int(np.ceil(max(max(nbs), 1) / 128.0)) * 128)
    T_pad = min(T_pad, S)

    in_maps = []
    for b in range(B):
        xs = np.ascontiguousarray(x[b].T[:, perms[b]]) * 16.0
        xsh = xs.astype(np.float16)
        xsl = (xs - xsh.astype(np.float32)).astype(np.float16)
        biasrow = np.zeros((1, T_pad), dtype=np.float16)
        biasrow[0, nbs[b]:] = BIAS_NEG
        in_maps.append({"xh": xsh, "xl": xsl, "w": w_cat, "biasrow": biasrow})

    import time as _time
    _t0 = _time.time()
    print(f"[kernel] building graph, T_pad={T_pad}", flush=True)
    nc = _build((bass, mybir, tile, bacc), T_pad)
    print(f"[kernel] graph+bacc compile done in {_time.time() - _t0:.1f}s",
          flush=True)

    trace = os.environ.get("BASS_KERNEL_TRACE", "0") == "1"
    if trace:
        import sys
        import types
        from trn_agent_boot.trn_boot import _ntff_profile_via_ctypes
        hook = _ntff_profile_via_ctypes("/opt/axon/libaxon_pjrt.so")
        m = types.ModuleType("antenv.axon_hooks")
        m.get_axon_ntff_profile_hook = lambda: hook
        sys.modules["antenv.axon_hooks"] = m
        bass_utils.upload_artifacts = lambda tmpdir: "local://" + tmpdir

    res = bass_utils.run_bass_kernel_spmd(
        nc, in_maps, core_ids=list(range(N_CORES)), trace=trace)
    if trace:
        print(f"HW exec time: {res.exec_time_ns} ns", flush=True)

    out = np.empty((B, S, H), dtype=np.float32)
    for b in range(B):
        o = res.results[b]["out"].reshape(128, 16, 16)[:, :, :H]
        out[b, perms[b], :] = o.transpose(1, 0, 2).reshape(S, H)
    return out
